# revision 3
# baseline (speedup 1.0000x reference)
"""Trainium2 Bass kernel for CustomEmbedding (embedding lookup with 16
override rows at the top of the vocab).

Semantics (matches the reference):
    out[b, s] = extra[input_ids[b, s] - 127984]  if input_ids[b, s] >= 127984
                weight[input_ids[b, s]]          otherwise

Sharding: data parallel over the batch dim — core c handles input_ids[c]
(4096 tokens), weight/extra replicated.

Per-core kernel (per tile of 128*K tokens):
    memset out_tile = 0
    out_tile  = gather(weight, ids)    with bounds_check=127983, OOB skipped
    out_tile += gather(extra, idx2)    idx2 = ids-127984 if reserved else BIG,
                                       bounds_check=15, OOB skipped, CCE add
    store out_tile -> out rows

The memset+add scheme is correct whether OOB gather positions are skipped
(HW DGE) or zero-filled (CoreSim).
"""

import sys

if "/opt/trn_rl_repo" not in sys.path:
    sys.path.insert(0, "/opt/trn_rl_repo")

import numpy as np

import concourse.bass as bass
import concourse.tile as tile
from concourse import bacc, mybir
from concourse.bass_utils import run_bass_kernel_spmd

VOCAB = 128000
DIM = 2048
B, S = 8, 4096
N_CORES = 8
N_OVER = 16
OVER_START = VOCAB - N_OVER  # 127984
P = 128
# Out-of-bounds marker index for the extra-table pass. Must stay < 2^20 so
# that BIG * DIM (the flat element offset) fits in int32 without wrapping —
# both CoreSim and the Q7 DGE compute the offset in 32-bit arithmetic, and a
# wrapped product can alias back into the table and dodge the bounds check.
BIG = 500_000

# Tunables
# K > 1 (more than 128 offsets per indirect DMA) crashes the runtime —
# keep one offset per partition per instruction.
K = 1
DATA_BUFS = 3

_NC_CACHE = {}


def _build_nc(k=K, data_bufs=DATA_BUFS):
    key = (k, data_bufs)
    if key in _NC_CACHE:
        return _NC_CACHE[key]

    cols = S // P  # 32 ids per partition
    n_tiles = cols // k

    nc = bacc.Bacc("TRN2", target_bir_lowering=False, debug=False)
    ids = nc.dram_tensor("input_ids", [S], mybir.dt.int32, kind="ExternalInput")
    weight = nc.dram_tensor(
        "weight", [VOCAB, DIM], mybir.dt.float32, kind="ExternalInput"
    )
    extra = nc.dram_tensor(
        "extra", [N_OVER, DIM], mybir.dt.float32, kind="ExternalInput"
    )
    out = nc.dram_tensor("out", [S, DIM], mybir.dt.float32, kind="ExternalOutput")

    # out viewed as [P, cols*DIM]: token tau = p*cols + c lives at partition p,
    # free-dim block c.
    out_view = out.ap().rearrange("(p c) d -> p (c d)", p=P)

    with tile.TileContext(nc) as tc:
        with (
            tc.tile_pool(name="idx", bufs=1) as idx_pool,
            tc.tile_pool(name="data", bufs=data_bufs) as data_pool,
        ):
            ids_sb = idx_pool.tile([P, cols], mybir.dt.int32)
            # contiguous: ids_sb[p, c] = ids[p*cols + c]
            nc.sync.dma_start(
                out=ids_sb[:], in_=ids.ap().rearrange("(p c) -> p c", p=P)
            )

            # idx2 = ids - OVER_START if ids >= OVER_START else BIG
            m = idx_pool.tile([P, cols], mybir.dt.int32)
            u = idx_pool.tile([P, cols], mybir.dt.int32)
            idx2 = idx_pool.tile([P, cols], mybir.dt.int32)
            nc.vector.tensor_scalar(
                m[:], ids_sb[:], OVER_START, None, mybir.AluOpType.is_ge
            )
            nc.vector.tensor_scalar(
                u[:], ids_sb[:], OVER_START + BIG, None, mybir.AluOpType.subtract
            )
            nc.vector.tensor_tensor(idx2[:], m[:], u[:], mybir.AluOpType.mult)
            nc.vector.tensor_scalar(
                idx2[:], idx2[:], BIG, None, mybir.AluOpType.add
            )

            for t in range(n_tiles):
                ot = data_pool.tile([P, k * DIM], mybir.dt.float32)
                nc.vector.memset(ot[:], 0.0)
                nc.gpsimd.indirect_dma_start(
                    out=ot[:],
                    out_offset=None,
                    in_=weight.ap(),
                    in_offset=bass.IndirectOffsetOnAxis(
                        ap=ids_sb[:, t * k : (t + 1) * k], axis=0
                    ),
                    bounds_check=OVER_START - 1,
                    oob_is_err=False,
                )
                nc.gpsimd.indirect_dma_start(
                    out=ot[:],
                    out_offset=None,
                    in_=extra.ap(),
                    in_offset=bass.IndirectOffsetOnAxis(
                        ap=idx2[:, t * k : (t + 1) * k], axis=0
                    ),
                    bounds_check=N_OVER - 1,
                    oob_is_err=False,
                    compute_op=mybir.AluOpType.add,
                )
                nc.sync.dma_start(
                    out=out_view[:, t * k * DIM : (t + 1) * k * DIM], in_=ot[:]
                )

    nc.compile()
    _NC_CACHE[key] = nc
    return nc


def kernel(input_ids, weight, extra):
    input_ids = np.ascontiguousarray(np.asarray(input_ids), dtype=np.int32)
    weight = np.ascontiguousarray(np.asarray(weight), dtype=np.float32)
    extra = np.ascontiguousarray(np.asarray(extra), dtype=np.float32)
    assert input_ids.shape == (B, S), input_ids.shape
    assert weight.shape == (VOCAB, DIM), weight.shape
    assert extra.shape == (N_OVER, DIM), extra.shape

    nc = _build_nc()
    in_maps = [
        {"input_ids": input_ids[c], "weight": weight, "extra": extra}
        for c in range(N_CORES)
    ]
    res = run_bass_kernel_spmd(nc, in_maps, core_ids=list(range(N_CORES)))
    return np.stack([res.results[c]["out"] for c in range(N_CORES)], axis=0)
